# revision 14
# baseline (speedup 1.0000x reference)
"""Trainium2 Bass kernel for BatchedCauchyKernel_CONCERT_flex.

Full-input contract: kernel(**inputs) takes the complete (unsharded)
numpy arrays, shards x/sample_x/cutoff rows across 8 NeuronCores
(data-parallel over the N axis of the output), replicates y/sample_y/
scale, and gathers the per-core [512, 4096] tiles into the full
[4096, 4096] output.

Math (reference):
    s        = clip(scale, 1e-6, 1e6)
    scale_x  = clip(sample_x @ s, 1e-6)        x_s = x / sqrt(scale_x)
    scale_y  = clip(sample_y @ s, 1e-6)        y_s = y / sqrt(scale_y)
    d        = clip(|x_s_i|^2 + |y_s_j|^2 - 2 x_s_i . y_s_j, 1e-6)
    res      = 1 / (1 + d)
    c        = clip(cutoff, 1e-4, 0.9999)
    cm_ij    = (c_i + c_j) / 2
    out      = res * sigmoid(clip(res - cm, -1, 1))     (iff mean(cutoff) > 0)

Device-side formulation (per core, rows i in a 512-row slice, tiles of
[128 x 1024] = 2 PSUM banks):
    PSUM tile = 1 + d  via accumulating matmuls per 512-wide half:
        (a) x_sT[128, 128chunk].T @ (-2 y_sT)[128, 512]     bf16 (K = D = 128)
        (b) split-bf16 aug: [x2h; x2l; 1; 1].T @ [1; 1; (1+y2)h; (1+y2)l]
            (K = 4, ~2^-16 effective precision on the x2 + y2 + 1 terms)
    res  = reciprocal_approx_fast(PSUM) -> bf16           DVE (1x, PSUM read)
    t2   = res - 0.5 c_j  (broadcast tile, bf16)          DVE (2x)
    mask = Sigmoid(t2 + bias),  bias = -0.5 c_i           ACT (per-part bias)
    out  = res * mask -> bf16, upcast to f32 on host      DVE (2x)
    All epilogue tensor ops stay on DVE: gpsimd 2-input ops share DVE's
    SBUF ports and stall concurrent DVE work 2-4x (measured).
    An 8-matmul zero warmup trips the PE HAM clock gate toward 2.4 GHz
    during the input-DMA window.
    clip(d, 1e-6) and clip(res - cm, -1, 1) are provably no-ops here
    (d >> 1e-6 at this operand scale; 0 < res <= 1 and 1e-4 <= cm <= 1
    imply res - cm strictly inside (-1, 1)).
The row scaling / row norms (O(N*D), 0.025% of the FLOPs) are host prep.
"""

from __future__ import annotations

import numpy as np

N = 4096
D = 128
S = 16
NCORES = 8
R = N // NCORES          # 512 rows of x per core
RCHUNKS = R // 128       # 4 row chunks of 128 (PSUM partition dim)
W = 1024                 # epilogue tile width (2 PSUM banks)
CCHUNKS = N // W         # 4 column chunks per core

_PROGRAM_CACHE = {}


def _build_program(apply_gate: bool):
    from contextlib import ExitStack

    import concourse.bass as bass
    import concourse.tile as tile
    from concourse import bacc, mybir

    from concourse.dve_ops import RECIP_APPROX_FAST_CONSTS, RECIPROCAL_APPROX_FAST

    f32 = mybir.dt.float32
    f32r = mybir.dt.float32r
    bf16 = mybir.dt.bfloat16
    alu = mybir.AluOpType
    rc = RECIP_APPROX_FAST_CONSTS

    nc = bacc.Bacc()

    xsT_d = nc.declare_dram_parameter("xsT", [128, R], bf16, isOutput=False)
    ysT_d = nc.declare_dram_parameter("ysT", [128, N], bf16, isOutput=False)
    augx_d = nc.declare_dram_parameter("augx", [4, R], bf16, isOutput=False)
    augy_d = nc.declare_dram_parameter("augy", [4, N], bf16, isOutput=False)
    nhci_d = nc.declare_dram_parameter("nhci", [128, RCHUNKS], f32, isOutput=False)
    hcj_d = nc.declare_dram_parameter("hcj", [1, N], bf16, isOutput=False)
    out_d = nc.declare_dram_parameter("out", [R, N], bf16, isOutput=True)

    with ExitStack() as ctx:
        tc = ctx.enter_context(tile.TileContext(nc))
        consts = ctx.enter_context(tc.tile_pool(name="consts", bufs=1))
        dpsum = ctx.enter_context(tc.tile_pool(name="dpsum", bufs=4, space="PSUM"))
        work = ctx.enter_context(tc.tile_pool(name="work", bufs=8))

        # PE warmup: ~7us of dummy matmuls issued during the input-DMA
        # window so the HAM clock gate reaches 8/8 before the real work.
        wsrc = consts.tile([128, 512], bf16)
        nc.vector.memset(wsrc, 0.0)
        wp = dpsum.tile([128, W], f32, tag="d")
        for _ in range(8):
            nc.tensor.matmul(wp[:, 0:512], lhsT=wsrc[:, 0:128], rhs=wsrc, start=True, stop=True)

        xsT = consts.tile([128, R], bf16)
        for q in range(2):
            qs = slice(q * 256, (q + 1) * 256)
            nc.sync.dma_start(out=xsT[:, qs], in_=xsT_d[:, qs])
        augx = consts.tile([4, R], bf16)
        nc.sync.dma_start(out=augx, in_=augx_d[:, :])
        augy = consts.tile([4, N], bf16)
        nc.sync.dma_start(out=augy, in_=augy_d[:, :])
        ysT = consts.tile([128, N], bf16)
        for q in range(16):
            qs = slice(q * 256, (q + 1) * 256)
            nc.sync.dma_start(out=ysT[:, qs], in_=ysT_d[:, qs])
        if apply_gate:
            nhci = consts.tile([128, RCHUNKS], f32)
            nc.sync.dma_start(out=nhci, in_=nhci_d[:, :])
            # cjb[p, j] = 0.5*c_j, partition-broadcast straight from DRAM.
            cjb = consts.tile([128, N], bf16)
            for q in range(8):
                qs = slice(q * 512, (q + 1) * 512)
                src = hcj_d[0:1, qs]
                src_b = bass.AP(
                    tensor=src.tensor,
                    offset=src.offset,
                    ap=[[0, 128], src.ap[-1]],
                )
                nc.sync.dma_start(out=cjb[:, qs], in_=src_b)

        for c in range(CCHUNKS):
            cs = slice(c * W, (c + 1) * W)
            for r in range(RCHUNKS):
                rs = slice(r * 128, (r + 1) * 128)
                pd = dpsum.tile([128, W], f32, tag="d")
                # Same-lhsT matmuls grouped: halves weight switches on PE.
                for h in range(2):
                    hs = slice(c * W + h * 512, c * W + (h + 1) * 512)
                    ps = slice(h * 512, (h + 1) * 512)
                    nc.tensor.matmul(
                        pd[:, ps],
                        lhsT=xsT[:, rs],
                        rhs=ysT[:, hs],
                        start=True,
                        stop=False,
                    )
                for h in range(2):
                    hs = slice(c * W + h * 512, c * W + (h + 1) * 512)
                    ps = slice(h * 512, (h + 1) * 512)
                    nc.tensor.matmul(
                        pd[:, ps],
                        lhsT=augx[:, rs],
                        rhs=augy[:, hs],
                        start=False,
                        stop=True,
                    )
                t = c * RCHUNKS + r
                res = work.tile([128, W], bf16, tag="res")
                nc.vector._custom_dve(
                    RECIPROCAL_APPROX_FAST,
                    out=res,
                    in0=pd,
                    s0=rc["s0"],
                    s1=rc["s1"],
                    imm2=rc["imm2"],
                )
                if apply_gate:
                    # Keep ALL tensor ops on DVE: gpsimd 2-input ops share
                    # DVE's SBUF ports and slow concurrent DVE ops 2-4x.
                    t2 = work.tile([128, W], bf16, tag="t2")
                    nc.vector.tensor_tensor(
                        out=t2, in0=res, in1=cjb[:, cs], op=alu.subtract
                    )
                    mask = work.tile([128, W], bf16, tag="mask")
                    nc.scalar.activation(
                        out=mask,
                        in_=t2,
                        func=mybir.ActivationFunctionType.Sigmoid,
                        bias=nhci[:, r : r + 1],
                    )
                    ot = work.tile([128, W], bf16, tag="ot")
                    nc.vector.tensor_tensor(out=ot, in0=res, in1=mask, op=alu.mult)
                else:
                    ot = res
                # Two HWDGE writes per tile: the final tile's 256KB drains on
                # two queues (~5.6us) instead of one (~11us).
                lo = slice(c * W, c * W + 512)
                hi = slice(c * W + 512, (c + 1) * W)
                nc.sync.dma_start(out=out_d[rs, lo], in_=ot[:, 0:512])
                nc.sync.dma_start(out=out_d[rs, hi], in_=ot[:, 512:1024])

    nc.finalize()
    return nc


def kernel(x, y, sample_x, sample_y, scale, cutoff):
    import ml_dtypes

    from concourse.bass_utils import run_bass_kernel_spmd

    f32 = np.float32
    bf16 = ml_dtypes.bfloat16

    # Host prep in float64 for accuracy, cast down for the device.
    x64 = np.asarray(x, np.float64)
    y64 = np.asarray(y, np.float64)
    s64 = np.clip(np.asarray(scale, np.float64), 1e-6, 1e6)
    scale_x = np.clip(np.asarray(sample_x, np.float64) @ s64, 1e-6, None)
    scale_y = np.clip(np.asarray(sample_y, np.float64) @ s64, 1e-6, None)
    x_s = (x64 / np.sqrt(scale_x)).astype(f32)          # [N, D]
    y_s = (y64 / np.sqrt(scale_y)).astype(f32)          # [N, D]
    # Norms from the bf16-rounded operands the PE will actually multiply,
    # so the x2/y2 terms match the -2xy term's operand rounding.
    x_sb = x_s.astype(bf16)
    y_sb = y_s.astype(bf16)
    x2 = np.sum(x_sb.astype(np.float64) ** 2, axis=1)   # [N]
    y2 = np.sum(y_sb.astype(np.float64) ** 2, axis=1)   # [N]

    ysT = np.ascontiguousarray((-2.0 * y_sb.astype(np.float64)).T).astype(bf16)
    xsT_full = np.ascontiguousarray(x_sb.T)                      # [128, N] bf16
    y2p1 = y2 + 1.0
    yh = y2p1.astype(bf16)
    yl = (y2p1 - yh.astype(np.float64)).astype(bf16)
    ones_n = np.ones(N, np.float64)
    augy = np.ascontiguousarray(
        np.stack([ones_n, ones_n, yh.astype(np.float64), yl.astype(np.float64)])
    ).astype(bf16)                                               # [4, N]
    x2h = x2.astype(bf16)
    x2l = (x2 - x2h.astype(np.float64)).astype(bf16)
    c_half = 0.5 * np.clip(np.asarray(cutoff, np.float64), 1e-4, 0.9999)
    hcj = np.ascontiguousarray(c_half.reshape(1, N)).astype(bf16)  # [1, N]

    apply_gate = bool(np.mean(np.asarray(cutoff, np.float64)) > 0.0)

    key = apply_gate
    if key not in _PROGRAM_CACHE:
        _PROGRAM_CACHE[key] = _build_program(apply_gate)
    nc = _PROGRAM_CACHE[key]

    in_maps = []
    for i in range(NCORES):
        rows = slice(i * R, (i + 1) * R)
        ones_r = np.ones(R, np.float64)
        augx = np.ascontiguousarray(
            np.stack(
                [x2h.astype(np.float64)[rows], x2l.astype(np.float64)[rows],
                 ones_r, ones_r]
            )
        ).astype(bf16)                                           # [4, R]
        nhci = np.ascontiguousarray(
            -c_half[rows, 0].reshape(RCHUNKS, 128).T, dtype=f32
        )                                                        # [128, RCHUNKS]
        in_maps.append(
            {
                "xsT": np.ascontiguousarray(xsT_full[:, rows]),
                "ysT": ysT,
                "augx": augx,
                "augy": augy,
                "nhci": nhci,
                "hcj": hcj,
            }
        )

    out = run_bass_kernel_spmd(nc, in_maps, list(range(NCORES)))
    full = np.concatenate(
        [np.asarray(out.results[i]["out"]) for i in range(NCORES)], axis=0
    )
    return np.ascontiguousarray(full.astype(f32))


# revision 15
# speedup vs baseline: 1.0220x; 1.0220x over previous
"""Trainium2 Bass kernel for BatchedCauchyKernel_CONCERT_flex.

Full-input contract: kernel(**inputs) takes the complete (unsharded)
numpy arrays, shards x/sample_x/cutoff rows across 8 NeuronCores
(data-parallel over the N axis of the output), replicates y/sample_y/
scale, and gathers the per-core [512, 4096] tiles into the full
[4096, 4096] output.

Math (reference):
    s        = clip(scale, 1e-6, 1e6)
    scale_x  = clip(sample_x @ s, 1e-6)        x_s = x / sqrt(scale_x)
    scale_y  = clip(sample_y @ s, 1e-6)        y_s = y / sqrt(scale_y)
    d        = clip(|x_s_i|^2 + |y_s_j|^2 - 2 x_s_i . y_s_j, 1e-6)
    res      = 1 / (1 + d)
    c        = clip(cutoff, 1e-4, 0.9999)
    cm_ij    = (c_i + c_j) / 2
    out      = res * sigmoid(clip(res - cm, -1, 1))     (iff mean(cutoff) > 0)

Device-side formulation (per core, rows i in a 512-row slice, tiles of
[128 x 1024] = 2 PSUM banks):
    PSUM tile = 1 + d  via accumulating matmuls per 512-wide half:
        (a) x_sT[128, 128chunk].T @ (-2 y_sT)[128, 512]     bf16 (K = D = 128)
        (b) split-bf16 aug: [x2h; x2l; 1; 1].T @ [1; 1; (1+y2)h; (1+y2)l]
            (K = 4, ~2^-16 effective precision on the x2 + y2 + 1 terms)
    res  = reciprocal_approx_fast(PSUM) -> bf16           DVE (1x, PSUM read)
    t2   = res - 0.5 c_j  (broadcast tile, bf16)          DVE (2x)
    mask = Sigmoid(t2 + bias),  bias = -0.5 c_i           ACT (per-part bias)
    out  = res * mask -> bf16, upcast to f32 on host      DVE (2x)
    All epilogue tensor ops stay on DVE: gpsimd 2-input ops share DVE's
    SBUF ports and stall concurrent DVE work 2-4x (measured).
    An 8-matmul zero warmup trips the PE HAM clock gate toward 2.4 GHz
    during the input-DMA window.
    clip(d, 1e-6) and clip(res - cm, -1, 1) are provably no-ops here
    (d >> 1e-6 at this operand scale; 0 < res <= 1 and 1e-4 <= cm <= 1
    imply res - cm strictly inside (-1, 1)).
The row scaling / row norms (O(N*D), 0.025% of the FLOPs) are host prep.
"""

from __future__ import annotations

import numpy as np

N = 4096
D = 128
S = 16
NCORES = 8
R = N // NCORES          # 512 rows of x per core
RCHUNKS = R // 128       # 4 row chunks of 128 (PSUM partition dim)
W = 1024                 # epilogue tile width (2 PSUM banks)
CCHUNKS = N // W         # 4 column chunks per core

_PROGRAM_CACHE = {}


def _build_program(apply_gate: bool):
    from contextlib import ExitStack

    import concourse.bass as bass
    import concourse.tile as tile
    from concourse import bacc, mybir

    from concourse.dve_ops import RECIP_APPROX_FAST_CONSTS, RECIPROCAL_APPROX_FAST

    f32 = mybir.dt.float32
    f32r = mybir.dt.float32r
    bf16 = mybir.dt.bfloat16
    alu = mybir.AluOpType
    rc = RECIP_APPROX_FAST_CONSTS

    nc = bacc.Bacc()

    xsT_d = nc.declare_dram_parameter("xsT", [128, R], bf16, isOutput=False)
    ysT_d = nc.declare_dram_parameter("ysT", [128, N], bf16, isOutput=False)
    augx_d = nc.declare_dram_parameter("augx", [4, R], bf16, isOutput=False)
    augy_d = nc.declare_dram_parameter("augy", [4, N], bf16, isOutput=False)
    nhci_d = nc.declare_dram_parameter("nhci", [128, RCHUNKS], f32, isOutput=False)
    hcj_d = nc.declare_dram_parameter("hcj", [1, N], bf16, isOutput=False)
    out_d = nc.declare_dram_parameter("out", [R, N], bf16, isOutput=True)

    with ExitStack() as ctx:
        tc = ctx.enter_context(tile.TileContext(nc))
        consts = ctx.enter_context(tc.tile_pool(name="consts", bufs=1))
        dpsum = ctx.enter_context(tc.tile_pool(name="dpsum", bufs=4, space="PSUM"))
        work = ctx.enter_context(tc.tile_pool(name="work", bufs=8))

        # PE warmup: ~7us of dummy matmuls issued during the input-DMA
        # window so the HAM clock gate reaches 8/8 before the real work.
        wsrc = consts.tile([128, 512], bf16)
        nc.vector.memset(wsrc, 0.0)
        wp = dpsum.tile([128, W], f32, tag="d")
        for _ in range(8):
            nc.tensor.matmul(wp[:, 0:512], lhsT=wsrc[:, 0:128], rhs=wsrc, start=True, stop=True)

        xsT = consts.tile([128, R], bf16)
        nc.sync.dma_start(out=xsT, in_=xsT_d[:, :])
        augx = consts.tile([4, R], bf16)
        nc.sync.dma_start(out=augx, in_=augx_d[:, :])
        augy = consts.tile([4, N], bf16)
        nc.sync.dma_start(out=augy, in_=augy_d[:, :])
        ysT = consts.tile([128, N], bf16)
        for q in range(8):
            qs = slice(q * 512, (q + 1) * 512)
            nc.sync.dma_start(out=ysT[:, qs], in_=ysT_d[:, qs])
        if apply_gate:
            nhci = consts.tile([128, RCHUNKS], f32)
            nc.sync.dma_start(out=nhci, in_=nhci_d[:, :])
            # cjb[p, j] = 0.5*c_j, partition-broadcast straight from DRAM.
            cjb = consts.tile([128, N], bf16)
            for q in range(8):
                qs = slice(q * 512, (q + 1) * 512)
                src = hcj_d[0:1, qs]
                src_b = bass.AP(
                    tensor=src.tensor,
                    offset=src.offset,
                    ap=[[0, 128], src.ap[-1]],
                )
                nc.sync.dma_start(out=cjb[:, qs], in_=src_b)

        for c in range(CCHUNKS):
            cs = slice(c * W, (c + 1) * W)
            for r in range(RCHUNKS):
                rs = slice(r * 128, (r + 1) * 128)
                pd = dpsum.tile([128, W], f32, tag="d")
                # Same-lhsT matmuls grouped: halves weight switches on PE.
                for h in range(2):
                    hs = slice(c * W + h * 512, c * W + (h + 1) * 512)
                    ps = slice(h * 512, (h + 1) * 512)
                    nc.tensor.matmul(
                        pd[:, ps],
                        lhsT=xsT[:, rs],
                        rhs=ysT[:, hs],
                        start=True,
                        stop=False,
                    )
                for h in range(2):
                    hs = slice(c * W + h * 512, c * W + (h + 1) * 512)
                    ps = slice(h * 512, (h + 1) * 512)
                    nc.tensor.matmul(
                        pd[:, ps],
                        lhsT=augx[:, rs],
                        rhs=augy[:, hs],
                        start=False,
                        stop=True,
                    )
                t = c * RCHUNKS + r
                res = work.tile([128, W], bf16, tag="res")
                nc.vector._custom_dve(
                    RECIPROCAL_APPROX_FAST,
                    out=res,
                    in0=pd,
                    s0=rc["s0"],
                    s1=rc["s1"],
                    imm2=rc["imm2"],
                )
                if apply_gate:
                    # Keep ALL tensor ops on DVE: gpsimd 2-input ops share
                    # DVE's SBUF ports and slow concurrent DVE ops 2-4x.
                    t2 = work.tile([128, W], bf16, tag="t2")
                    nc.vector.tensor_tensor(
                        out=t2, in0=res, in1=cjb[:, cs], op=alu.subtract
                    )
                    mask = work.tile([128, W], bf16, tag="mask")
                    nc.scalar.activation(
                        out=mask,
                        in_=t2,
                        func=mybir.ActivationFunctionType.Sigmoid,
                        bias=nhci[:, r : r + 1],
                    )
                    ot = work.tile([128, W], bf16, tag="ot")
                    nc.vector.tensor_tensor(out=ot, in0=res, in1=mask, op=alu.mult)
                else:
                    ot = res
                # Split writeback: lo half HWDGE (sync), hi half SWDGE on the
                # otherwise-idle gpsimd engine. Two queues + two sequencers, so
                # the final tile drains in ~6us instead of ~11us, without
                # loading the Sync sequencer with 2x triggers.
                lo = slice(c * W, c * W + 512)
                hi = slice(c * W + 512, (c + 1) * W)
                nc.sync.dma_start(out=out_d[rs, lo], in_=ot[:, 0:512])
                nc.gpsimd.dma_start(out=out_d[rs, hi], in_=ot[:, 512:1024])

    nc.finalize()
    return nc


def kernel(x, y, sample_x, sample_y, scale, cutoff):
    import ml_dtypes

    from concourse.bass_utils import run_bass_kernel_spmd

    f32 = np.float32
    bf16 = ml_dtypes.bfloat16

    # Host prep in float64 for accuracy, cast down for the device.
    x64 = np.asarray(x, np.float64)
    y64 = np.asarray(y, np.float64)
    s64 = np.clip(np.asarray(scale, np.float64), 1e-6, 1e6)
    scale_x = np.clip(np.asarray(sample_x, np.float64) @ s64, 1e-6, None)
    scale_y = np.clip(np.asarray(sample_y, np.float64) @ s64, 1e-6, None)
    x_s = (x64 / np.sqrt(scale_x)).astype(f32)          # [N, D]
    y_s = (y64 / np.sqrt(scale_y)).astype(f32)          # [N, D]
    # Norms from the bf16-rounded operands the PE will actually multiply,
    # so the x2/y2 terms match the -2xy term's operand rounding.
    x_sb = x_s.astype(bf16)
    y_sb = y_s.astype(bf16)
    x2 = np.sum(x_sb.astype(np.float64) ** 2, axis=1)   # [N]
    y2 = np.sum(y_sb.astype(np.float64) ** 2, axis=1)   # [N]

    ysT = np.ascontiguousarray((-2.0 * y_sb.astype(np.float64)).T).astype(bf16)
    xsT_full = np.ascontiguousarray(x_sb.T)                      # [128, N] bf16
    y2p1 = y2 + 1.0
    yh = y2p1.astype(bf16)
    yl = (y2p1 - yh.astype(np.float64)).astype(bf16)
    ones_n = np.ones(N, np.float64)
    augy = np.ascontiguousarray(
        np.stack([ones_n, ones_n, yh.astype(np.float64), yl.astype(np.float64)])
    ).astype(bf16)                                               # [4, N]
    x2h = x2.astype(bf16)
    x2l = (x2 - x2h.astype(np.float64)).astype(bf16)
    c_half = 0.5 * np.clip(np.asarray(cutoff, np.float64), 1e-4, 0.9999)
    hcj = np.ascontiguousarray(c_half.reshape(1, N)).astype(bf16)  # [1, N]

    apply_gate = bool(np.mean(np.asarray(cutoff, np.float64)) > 0.0)

    key = apply_gate
    if key not in _PROGRAM_CACHE:
        _PROGRAM_CACHE[key] = _build_program(apply_gate)
    nc = _PROGRAM_CACHE[key]

    in_maps = []
    for i in range(NCORES):
        rows = slice(i * R, (i + 1) * R)
        ones_r = np.ones(R, np.float64)
        augx = np.ascontiguousarray(
            np.stack(
                [x2h.astype(np.float64)[rows], x2l.astype(np.float64)[rows],
                 ones_r, ones_r]
            )
        ).astype(bf16)                                           # [4, R]
        nhci = np.ascontiguousarray(
            -c_half[rows, 0].reshape(RCHUNKS, 128).T, dtype=f32
        )                                                        # [128, RCHUNKS]
        in_maps.append(
            {
                "xsT": np.ascontiguousarray(xsT_full[:, rows]),
                "ysT": ysT,
                "augx": augx,
                "augy": augy,
                "nhci": nhci,
                "hcj": hcj,
            }
        )

    out = run_bass_kernel_spmd(nc, in_maps, list(range(NCORES)))
    full = np.concatenate(
        [np.asarray(out.results[i]["out"]) for i in range(NCORES)], axis=0
    )
    return np.ascontiguousarray(full.astype(f32))


# revision 16
# speedup vs baseline: 1.0432x; 1.0207x over previous
"""Trainium2 Bass kernel for BatchedCauchyKernel_CONCERT_flex.

Full-input contract: kernel(**inputs) takes the complete (unsharded)
numpy arrays, shards x/sample_x/cutoff rows across 8 NeuronCores
(data-parallel over the N axis of the output), replicates y/sample_y/
scale, and gathers the per-core [512, 4096] tiles into the full
[4096, 4096] output.

Math (reference):
    s        = clip(scale, 1e-6, 1e6)
    scale_x  = clip(sample_x @ s, 1e-6)        x_s = x / sqrt(scale_x)
    scale_y  = clip(sample_y @ s, 1e-6)        y_s = y / sqrt(scale_y)
    d        = clip(|x_s_i|^2 + |y_s_j|^2 - 2 x_s_i . y_s_j, 1e-6)
    res      = 1 / (1 + d)
    c        = clip(cutoff, 1e-4, 0.9999)
    cm_ij    = (c_i + c_j) / 2
    out      = res * sigmoid(clip(res - cm, -1, 1))     (iff mean(cutoff) > 0)

Device-side formulation (per core, rows i in a 512-row slice, tiles of
[128 x 1024] = 2 PSUM banks):
    PSUM tile = 1 + d  via accumulating matmuls per 512-wide half:
        (a) x_sT[128, 128chunk].T @ (-2 y_sT)[128, 512]     bf16 (K = D = 128)
        (b) split-bf16 aug: [x2h; x2l; 1; 1].T @ [1; 1; (1+y2)h; (1+y2)l]
            (K = 4, ~2^-16 effective precision on the x2 + y2 + 1 terms)
    res  = reciprocal_approx_fast(PSUM) -> bf16           DVE (1x, PSUM read)
    t2   = res - 0.5 c_j  (broadcast tile, bf16)          DVE (2x)
    mask = Sigmoid(t2 + bias),  bias = -0.5 c_i           ACT (per-part bias)
    out  = res * mask -> bf16, upcast to f32 on host      DVE (2x)
    All epilogue tensor ops stay on DVE: gpsimd 2-input ops share DVE's
    SBUF ports and stall concurrent DVE work 2-4x (measured).
    An 8-matmul zero warmup trips the PE HAM clock gate toward 2.4 GHz
    during the input-DMA window.
    clip(d, 1e-6) and clip(res - cm, -1, 1) are provably no-ops here
    (d >> 1e-6 at this operand scale; 0 < res <= 1 and 1e-4 <= cm <= 1
    imply res - cm strictly inside (-1, 1)).
The row scaling / row norms (O(N*D), 0.025% of the FLOPs) are host prep.
"""

from __future__ import annotations

import numpy as np

N = 4096
D = 128
S = 16
NCORES = 8
R = N // NCORES          # 512 rows of x per core
RCHUNKS = R // 128       # 4 row chunks of 128 (PSUM partition dim)
W = 1024                 # epilogue tile width (2 PSUM banks)
CCHUNKS = N // W         # 4 column chunks per core

_PROGRAM_CACHE = {}


def _build_program(apply_gate: bool):
    from contextlib import ExitStack

    import concourse.bass as bass
    import concourse.tile as tile
    from concourse import bacc, mybir

    from concourse.dve_ops import RECIP_APPROX_FAST_CONSTS, RECIPROCAL_APPROX_FAST

    f32 = mybir.dt.float32
    f32r = mybir.dt.float32r
    bf16 = mybir.dt.bfloat16
    alu = mybir.AluOpType
    rc = RECIP_APPROX_FAST_CONSTS

    nc = bacc.Bacc()

    xsT_d = nc.declare_dram_parameter("xsT", [128, R], bf16, isOutput=False)
    ysT_d = nc.declare_dram_parameter("ysT", [128, N], bf16, isOutput=False)
    augx_d = nc.declare_dram_parameter("augx", [4, R], bf16, isOutput=False)
    augy_d = nc.declare_dram_parameter("augy", [4, N], bf16, isOutput=False)
    nhci_d = nc.declare_dram_parameter("nhci", [128, RCHUNKS], f32, isOutput=False)
    hcj_d = nc.declare_dram_parameter("hcj", [1, N], bf16, isOutput=False)
    out_d = nc.declare_dram_parameter("out", [R, N], bf16, isOutput=True)

    with ExitStack() as ctx:
        tc = ctx.enter_context(tile.TileContext(nc))
        consts = ctx.enter_context(tc.tile_pool(name="consts", bufs=1))
        dpsum = ctx.enter_context(tc.tile_pool(name="dpsum", bufs=4, space="PSUM"))
        work = ctx.enter_context(tc.tile_pool(name="work", bufs=8))

        # PE warmup: ~7us of dummy matmuls issued during the input-DMA
        # window so the HAM clock gate reaches 8/8 before the real work.
        wsrc = consts.tile([128, 512], bf16)
        nc.vector.memset(wsrc, 0.0)
        wp = dpsum.tile([128, W], f32, tag="d")
        for _ in range(8):
            nc.tensor.matmul(wp[:, 0:512], lhsT=wsrc[:, 0:128], rhs=wsrc, start=True, stop=True)

        xsT = consts.tile([128, R], bf16)
        nc.sync.dma_start(out=xsT, in_=xsT_d[:, :])
        augx = consts.tile([4, R], bf16)
        nc.sync.dma_start(out=augx, in_=augx_d[:, :])
        augy = consts.tile([4, N], bf16)
        nc.sync.dma_start(out=augy, in_=augy_d[:, :])
        ysT = consts.tile([128, N], bf16)
        for q in range(8):
            qs = slice(q * 512, (q + 1) * 512)
            nc.sync.dma_start(out=ysT[:, qs], in_=ysT_d[:, qs])
        if apply_gate:
            nhci = consts.tile([128, RCHUNKS], f32)
            nc.sync.dma_start(out=nhci, in_=nhci_d[:, :])
            # cjb[p, j] = 0.5*c_j, partition-broadcast straight from DRAM.
            cjb = consts.tile([128, N], bf16)
            for q in range(8):
                qs = slice(q * 512, (q + 1) * 512)
                src = hcj_d[0:1, qs]
                src_b = bass.AP(
                    tensor=src.tensor,
                    offset=src.offset,
                    ap=[[0, 128], src.ap[-1]],
                )
                nc.sync.dma_start(out=cjb[:, qs], in_=src_b)

        for c in range(CCHUNKS):
            cs = slice(c * W, (c + 1) * W)
            for r in range(RCHUNKS):
                rs = slice(r * 128, (r + 1) * 128)
                pd = dpsum.tile([128, W], f32, tag="d")
                # Same-lhsT matmuls grouped: halves weight switches on PE.
                for h in range(2):
                    hs = slice(c * W + h * 512, c * W + (h + 1) * 512)
                    ps = slice(h * 512, (h + 1) * 512)
                    nc.tensor.matmul(
                        pd[:, ps],
                        lhsT=xsT[:, rs],
                        rhs=ysT[:, hs],
                        start=True,
                        stop=False,
                    )
                for h in range(2):
                    hs = slice(c * W + h * 512, c * W + (h + 1) * 512)
                    ps = slice(h * 512, (h + 1) * 512)
                    nc.tensor.matmul(
                        pd[:, ps],
                        lhsT=augx[:, rs],
                        rhs=augy[:, hs],
                        start=False,
                        stop=True,
                    )
                t = c * RCHUNKS + r
                res = work.tile([128, W], bf16, tag="res")
                nc.vector._custom_dve(
                    RECIPROCAL_APPROX_FAST,
                    out=res,
                    in0=pd,
                    s0=rc["s0"],
                    s1=rc["s1"],
                    imm2=rc["imm2"],
                )
                if apply_gate:
                    # Keep ALL tensor ops on DVE: gpsimd 2-input ops share
                    # DVE's SBUF ports and slow concurrent DVE ops 2-4x.
                    t2 = work.tile([128, W], bf16, tag="t2")
                    nc.vector.tensor_tensor(
                        out=t2, in0=res, in1=cjb[:, cs], op=alu.subtract
                    )
                    mask = work.tile([128, W], bf16, tag="mask")
                    nc.scalar.activation(
                        out=mask,
                        in_=t2,
                        func=mybir.ActivationFunctionType.Sigmoid,
                        bias=nhci[:, r : r + 1],
                    )
                    ot = work.tile([128, W], bf16, tag="ot")
                    nc.vector.tensor_tensor(out=ot, in0=res, in1=mask, op=alu.mult)
                else:
                    ot = res
                # Single HWDGE writeback per tile: one dma_start already fans
                # out across HW queues; gpsimd SWDGE is avoided entirely since
                # any gpsimd use costs ~6us of head (Q7 library load barrier).
                nc.sync.dma_start(out=out_d[rs, cs], in_=ot)

    nc.finalize()
    return nc


def kernel(x, y, sample_x, sample_y, scale, cutoff):
    import ml_dtypes

    from concourse.bass_utils import run_bass_kernel_spmd

    f32 = np.float32
    bf16 = ml_dtypes.bfloat16

    # Host prep in float64 for accuracy, cast down for the device.
    x64 = np.asarray(x, np.float64)
    y64 = np.asarray(y, np.float64)
    s64 = np.clip(np.asarray(scale, np.float64), 1e-6, 1e6)
    scale_x = np.clip(np.asarray(sample_x, np.float64) @ s64, 1e-6, None)
    scale_y = np.clip(np.asarray(sample_y, np.float64) @ s64, 1e-6, None)
    x_s = (x64 / np.sqrt(scale_x)).astype(f32)          # [N, D]
    y_s = (y64 / np.sqrt(scale_y)).astype(f32)          # [N, D]
    # Norms from the bf16-rounded operands the PE will actually multiply,
    # so the x2/y2 terms match the -2xy term's operand rounding.
    x_sb = x_s.astype(bf16)
    y_sb = y_s.astype(bf16)
    x2 = np.sum(x_sb.astype(np.float64) ** 2, axis=1)   # [N]
    y2 = np.sum(y_sb.astype(np.float64) ** 2, axis=1)   # [N]

    ysT = np.ascontiguousarray((-2.0 * y_sb.astype(np.float64)).T).astype(bf16)
    xsT_full = np.ascontiguousarray(x_sb.T)                      # [128, N] bf16
    y2p1 = y2 + 1.0
    yh = y2p1.astype(bf16)
    yl = (y2p1 - yh.astype(np.float64)).astype(bf16)
    ones_n = np.ones(N, np.float64)
    augy = np.ascontiguousarray(
        np.stack([ones_n, ones_n, yh.astype(np.float64), yl.astype(np.float64)])
    ).astype(bf16)                                               # [4, N]
    x2h = x2.astype(bf16)
    x2l = (x2 - x2h.astype(np.float64)).astype(bf16)
    c_half = 0.5 * np.clip(np.asarray(cutoff, np.float64), 1e-4, 0.9999)
    hcj = np.ascontiguousarray(c_half.reshape(1, N)).astype(bf16)  # [1, N]

    apply_gate = bool(np.mean(np.asarray(cutoff, np.float64)) > 0.0)

    key = apply_gate
    if key not in _PROGRAM_CACHE:
        _PROGRAM_CACHE[key] = _build_program(apply_gate)
    nc = _PROGRAM_CACHE[key]

    in_maps = []
    for i in range(NCORES):
        rows = slice(i * R, (i + 1) * R)
        ones_r = np.ones(R, np.float64)
        augx = np.ascontiguousarray(
            np.stack(
                [x2h.astype(np.float64)[rows], x2l.astype(np.float64)[rows],
                 ones_r, ones_r]
            )
        ).astype(bf16)                                           # [4, R]
        nhci = np.ascontiguousarray(
            -c_half[rows, 0].reshape(RCHUNKS, 128).T, dtype=f32
        )                                                        # [128, RCHUNKS]
        in_maps.append(
            {
                "xsT": np.ascontiguousarray(xsT_full[:, rows]),
                "ysT": ysT,
                "augx": augx,
                "augy": augy,
                "nhci": nhci,
                "hcj": hcj,
            }
        )

    out = run_bass_kernel_spmd(nc, in_maps, list(range(NCORES)))
    full = np.concatenate(
        [np.asarray(out.results[i]["out"]) for i in range(NCORES)], axis=0
    )
    return np.ascontiguousarray(full.astype(f32))
